# revision 1
# baseline (speedup 1.0000x reference)
"""GraphConv GNN kernel for trn2: host preprocessing + bass program builder.

Sharding: nodes (and incident edges, by dst) across 8 cores. Aggregation via
dma_gather (node-major bf16 rows) + one-hot matmul segment-sum. Weights
replicated. Per-layer AllGather of node features. Pooled partial sums + head
computed per-core, summed on host.
"""

import sys

sys.path.insert(0, "/opt/trn_rl_repo")

import numpy as np
import ml_dtypes

import os as _os
import concourse.bass as bass
import concourse.bacc as bacc
import concourse.tile as tile
import concourse.mybir as mybir
from concourse import library_config

BF16 = mybir.dt.bfloat16
F32 = mybir.dt.float32
I16 = mybir.dt.int16

N_CORES = 8
F = 128
N_CLASSES = 10

# per-window structure: K_LO lo-chunks + K_HI hi-chunks of 128 edges each
K_LO = 6
K_HI = 6
EDGES_PER_HALF = K_LO * 128  # 768
CHUNKS_PER_WIN = K_LO + K_HI
CPO = 32  # gather chunks per dma_gather op (4096 idxs)


def _wrap_idx(idx_flat):
    """idx i -> partition i%16, col i//16; replicated across the 8 Q7 core
    stripes (16 partitions each)."""
    n = idx_flat.shape[0]
    return np.ascontiguousarray(
        np.tile(idx_flat.reshape(n // 16, 16).T.astype(np.int16), (8, 1))
    )


def _wrap_ids(ids_flat):
    """edge e=c*128+p -> (p, c)."""
    n = ids_flat.shape[0]
    return np.ascontiguousarray(ids_flat.reshape(n // 128, 128).T.astype(np.float32))


def preprocess(x, edge_index, batch, params, n_nodes, n_graphs):
    """Build per-core inputs + meta for the SPMD program."""
    assert n_nodes % N_CORES == 0
    npc = n_nodes // N_CORES
    src = np.asarray(edge_index[0], np.int64)
    dst = np.asarray(edge_index[1], np.int64)
    batch = np.asarray(batch, np.int64)
    x = np.asarray(x, np.float32)

    half_node = (N_CORES // 2) * npc  # src < half_node -> "lo"

    # sort edges by dst once
    order = np.argsort(dst, kind="stable")
    src_s, dst_s = src[order], dst[order]

    # per-core edge ranges
    core_edge_start = np.searchsorted(dst_s, np.arange(0, n_nodes + 1, npc))

    # --- pass 1: greedy windows per core -> W_k, slots ---
    core_windows = []  # per core: list of (dst_start, dst_end) local
    for k in range(N_CORES):
        e0, e1 = core_edge_start[k], core_edge_start[k + 1]
        dl = dst_s[e0:e1] - k * npc
        sl_lo = src_s[e0:e1] < half_node
        deg_lo = np.bincount(dl[sl_lo], minlength=npc)
        deg_hi = np.bincount(dl[~sl_lo], minlength=npc)
        wins = []
        d = 0
        while d < npc:
            start = d
            lo = hi = 0
            while (
                d < npc
                and d - start < 128
                and lo + deg_lo[d] <= EDGES_PER_HALF
                and hi + deg_hi[d] <= EDGES_PER_HALF
            ):
                lo += deg_lo[d]
                hi += deg_hi[d]
                d += 1
            assert d > start, "single dst exceeds per-window edge budget"
            wins.append((start, d))
        core_windows.append(wins)

    w_star = max(len(w) for w in core_windows)
    w_star = (w_star + 3) // 4 * 4  # LOCAL_SLOTS multiple of 512
    ls = w_star * 128  # LOCAL_SLOTS
    rows = N_CORES * ls
    half_rows = rows // 2
    assert half_rows <= 32768, f"half_rows={half_rows} exceeds int16 idx range"

    # --- slots for every node ---
    slot = np.full(n_nodes, -1, np.int64)
    for k in range(N_CORES):
        for w, (a, b) in enumerate(core_windows[k]):
            d_loc = np.arange(a, b)
            slot[k * npc + d_loc] = w * 128 + (d_loc - a)
    assert (slot >= 0).all()
    owner = np.arange(n_nodes) // npc
    # node-major row index (transpose convention: fm pos s -> (p=s%128, c=s//128),
    # partition-major DRAM -> row = p*W* + c)
    row_of = owner * ls + (slot % 128) * w_star + slot // 128

    # --- per-core streams ---
    per_core = []
    sl_len = w_star * EDGES_PER_HALF  # per stream
    for k in range(N_CORES):
        e0, e1 = core_edge_start[k], core_edge_start[k + 1]
        dl = dst_s[e0:e1] - k * npc
        sv = src_s[e0:e1]
        is_lo = sv < half_node
        idx_lo = np.zeros((w_star, EDGES_PER_HALF), np.int64)
        ids_lo = np.full((w_star, EDGES_PER_HALF), -1.0, np.float32)
        idx_hi = np.zeros_like(idx_lo)
        ids_hi = np.full_like(ids_lo, -1.0)
        # edges are dst-sorted; window edge groups are contiguous
        wbounds = np.searchsorted(
            dl, [a for a, _ in core_windows[k]] + [npc]
        )
        for w, (a, b) in enumerate(core_windows[k]):
            lo_m = is_lo[wbounds[w] : wbounds[w + 1]]
            e_dst = dl[wbounds[w] : wbounds[w + 1]]
            e_src = sv[wbounds[w] : wbounds[w + 1]]
            for half, m in ((0, lo_m), (1, ~lo_m)):
                r = row_of[e_src[m]] - (0 if half == 0 else half_rows)
                cnt = r.shape[0]
                assert cnt <= EDGES_PER_HALF
                tgt_idx = idx_lo if half == 0 else idx_hi
                tgt_ids = ids_lo if half == 0 else ids_hi
                tgt_idx[w, :cnt] = r
                tgt_ids[w, :cnt] = (e_dst[m] - a).astype(np.float32)
        def _onehot(ids_arr):
            nch = ids_arr.size // 128
            ids_r = ids_arr.reshape(nch, 128)
            oh = (ids_r[:, :, None] == np.arange(128, dtype=np.float32)[None, None, :])
            return np.ascontiguousarray(
                oh.transpose(1, 0, 2).reshape(128, nch * 128).astype(ml_dtypes.bfloat16))

        per_core.append(
            dict(
                idx_lo=_wrap_idx(idx_lo.reshape(-1)),
                idx_hi=_wrap_idx(idx_hi.reshape(-1)),
                s_lo=_onehot(ids_lo.reshape(-1)),
                s_hi=_onehot(ids_hi.reshape(-1)),
            )
        )

    # --- x in both layouts ---
    x_bf = x.astype(ml_dtypes.bfloat16)
    x_full_nm = np.zeros((rows, F), ml_dtypes.bfloat16)
    x_full_nm[row_of] = x_bf

    in_maps = []
    for k in range(N_CORES):
        g = np.arange(k * npc, (k + 1) * npc)
        x_fm = np.zeros((F, ls), ml_dtypes.bfloat16)
        x_fm[:, slot[g]] = x_bf[g].T
        b_flat = np.full(ls, -1.0, np.float32)
        b_flat[slot[g]] = batch[g].astype(np.float32)
        batch_nm = b_flat.reshape(w_star, 128).T  # [p, c]
        b_onehot = (batch_nm[:, :, None] == np.arange(64, dtype=np.float32)[None, None, :])
        b_onehot = np.ascontiguousarray(
            b_onehot.reshape(128, w_star * 64).astype(ml_dtypes.bfloat16))
        m = dict(
            x_fm=x_fm,
            x_full_nm=x_full_nm,
            b_onehot=b_onehot,
            idx_lo=per_core[k]["idx_lo"],
            idx_hi=per_core[k]["idx_hi"],
            s_lo=per_core[k]["s_lo"],
            s_hi=per_core[k]["s_hi"],
            w1relT=np.ascontiguousarray(params["W1_rel"].T.astype(ml_dtypes.bfloat16)),
            w1rootT=np.ascontiguousarray(
                params["W1_root"].T.astype(ml_dtypes.bfloat16)
            ),
            w2relT=np.ascontiguousarray(params["W2_rel"].T.astype(ml_dtypes.bfloat16)),
            w2rootT=np.ascontiguousarray(
                params["W2_root"].T.astype(ml_dtypes.bfloat16)
            ),
            w3relT=np.ascontiguousarray(params["W3_rel"].T.astype(ml_dtypes.bfloat16)),
            w3rootT=np.ascontiguousarray(
                params["W3_root"].T.astype(ml_dtypes.bfloat16)
            ),
            b1=np.ascontiguousarray(params["b1_rel"].astype(np.float32).reshape(F, 1)),
            b2=np.ascontiguousarray(params["b2_rel"].astype(np.float32).reshape(F, 1)),
            b3=np.ascontiguousarray(params["b3_rel"].astype(np.float32).reshape(F, 1)),
            wlinT=np.ascontiguousarray(params["W_lin"].T.astype(np.float32)),
        )
        in_maps.append(m)

    meta = dict(w_star=w_star, ls=ls, rows=rows, half_rows=half_rows, n_graphs=n_graphs)
    return meta, in_maps


def build_nc(meta, n_graphs_pad=64):
    w_star = meta["w_star"]
    ls = meta["ls"]
    rows = meta["rows"]
    half_rows = meta["half_rows"]
    sl_len = w_star * EDGES_PER_HALF  # idxs per stream
    n_chunks = sl_len // 128
    dw = ls // 512  # dense windows
    ng = n_graphs_pad

    nc = bacc.Bacc(
        "TRN2", target_bir_lowering=False, debug=False, num_devices=N_CORES
    )

    # --- I/O ---
    x_fm_d = nc.dram_tensor("x_fm", [F, ls], BF16, kind="ExternalInput")
    x_full_d = nc.dram_tensor("x_full_nm", [rows, F], BF16, kind="ExternalInput")
    bone_d = nc.dram_tensor("b_onehot", [128, w_star * 64], BF16, kind="ExternalInput")
    idx_d = {
        "lo": nc.dram_tensor("idx_lo", [128, sl_len // 16], I16, kind="ExternalInput"),
        "hi": nc.dram_tensor("idx_hi", [128, sl_len // 16], I16, kind="ExternalInput"),
    }
    s_d = {
        "lo": nc.dram_tensor("s_lo", [128, n_chunks * 128], BF16, kind="ExternalInput"),
        "hi": nc.dram_tensor("s_hi", [128, n_chunks * 128], BF16, kind="ExternalInput"),
    }
    w_d = {}
    for l in (1, 2, 3):
        for p in ("rel", "root"):
            w_d[l, p] = nc.dram_tensor(f"w{l}{p}T", [F, F], BF16, kind="ExternalInput")
    b_d = {l: nc.dram_tensor(f"b{l}", [F, 1], F32, kind="ExternalInput") for l in (1, 2, 3)}
    wlin_d = nc.dram_tensor("wlinT", [F, N_CLASSES], F32, kind="ExternalInput")
    out_d = nc.dram_tensor("out_partial", [N_CLASSES, ng], F32, kind="ExternalOutput")

    relu = mybir.ActivationFunctionType.Relu
    ident = mybir.ActivationFunctionType.Identity
    copy_f = mybir.ActivationFunctionType.Copy

    with tile.TileContext(nc) as tc:
        with (
            tc.tile_pool(name="const", bufs=1) as constp,
            tc.tile_pool(name="state", bufs=1) as statep,
            tc.tile_pool(name="gpool", bufs=2) as gpool,
            tc.tile_pool(name="spool", bufs=8) as spool,
            tc.tile_pool(name="psa", bufs=2, space="PSUM") as psa,
            tc.tile_pool(name="psd", bufs=2, space="PSUM") as psd,
            tc.tile_pool(name="psp", bufs=1, space="PSUM") as psp,
            tc.tile_pool(name="dram", bufs=1, space="DRAM") as dramp,
        ):
            nc.gpsimd.load_library(library_config.mlp)

            # ---- load constants ----
            bone_t = constp.tile([128, w_star * 64], BF16)
            nc.sync.dma_start(bone_t[:], bone_d[:])
            idx_t = {}
            for h in ("lo", "hi"):
                it = constp.tile([128, sl_len // 16], I16, name=f"idx_{h}")
                nc.sync.dma_start(it[:], idx_d[h][:])
                idx_t[h] = it
            w_t = {}
            for key, d in w_d.items():
                wt = constp.tile([F, F], BF16, name=f"w_{key[0]}_{key[1]}")
                nc.sync.dma_start(wt[:], d[:])
                w_t[key] = wt
            b_t = {}
            for l, d in b_d.items():
                bt = constp.tile([F, 1], F32, name=f"b_{l}")
                nc.sync.dma_start(bt[:], d[:])
                b_t[l] = bt
            wlin_t = constp.tile([F, N_CLASSES], F32)
            nc.sync.dma_start(wlin_t[:], wlin_d[:])

            x_fm_t = statep.tile([F, ls], BF16, tag="h0")
            nc.sync.dma_start(x_fm_t[:], x_fm_d[:])

            # ---- layers ----
            h_fm = x_fm_t
            gather_src = x_full_d  # layer-1 source
            for layer in (1, 2, 3):
                # gather ops + S group loads
                g_tiles = {"lo": [], "hi": []}
                s_tiles = {"lo": [], "hi": []}
                n_ops = (n_chunks + CPO - 1) // CPO
                for o in range(n_ops):
                    c0 = o * CPO
                    c1 = min(n_chunks, c0 + CPO)
                    nch = c1 - c0
                    for h in ("lo", "hi"):
                        st_ = spool.tile(
                            [128, nch * 128],
                            BF16,
                            name=f"sg_{layer}_{h}_{o}",
                            tag=f"sg_{h}",
                            bufs=2,
                        )
                        nc.sync.dma_start(
                            st_[:], s_d[h][:, c0 * 128 : c1 * 128]
                        )
                        s_tiles[h].append(st_)
                        src_ap = (
                            gather_src[0:half_rows, :]
                            if h == "lo"
                            else gather_src[half_rows:rows, :]
                        )
                        gt = gpool.tile(
                            [128, nch, F],
                            BF16,
                            name=f"g_{layer}_{h}_{o}",
                            tag=f"g_{h}",
                            padded_shape=[128, CPO, F],
                        )
                        nidx = nch * 128
                        if True:
                            nc.gpsimd.dma_gather(
                                gt[:],
                                src_ap,
                                idx_t[h][:, c0 * 8 : c1 * 8],
                                nidx,
                                nidx,
                                F,
                                single_packet=False,
                            )
                        g_tiles[h].append(gt)

                # aggregation windows
                agg_fm = statep.tile([F, ls], BF16, tag=f"agg{layer % 2}", name=f"agg_{layer}")
                for w in range(w_star):
                    ps = psa.tile([128, 128], F32, name=f"psagg_{layer}_{w}", tag="psagg")
                    for j in range(CHUNKS_PER_WIN):
                        h = "lo" if j < K_LO else "hi"
                        cc = w * K_LO + (j % K_LO)
                        o, sl_ = cc // CPO, cc % CPO
                        nc.tensor.matmul(
                            ps[:],
                            g_tiles[h][o][:, sl_, :],
                            s_tiles[h][o][:, sl_ * 128 : (sl_ + 1) * 128],
                            start=(j == 0),
                            stop=(j == CHUNKS_PER_WIN - 1),
                        )
                    nc.scalar.activation(
                        agg_fm[:, w * 128 : (w + 1) * 128], ps[:], copy_f
                    )

                # dense
                h_next = statep.tile([F, ls], BF16, tag=f"h{layer}", name=f"h_{layer}")
                for d in range(dw):
                    ps = psd.tile([128, 512], F32, name=f"psd_{layer}_{d}", tag="psd")
                    sl2 = slice(d * 512, (d + 1) * 512)
                    nc.tensor.matmul(
                        ps[:], w_t[layer, "rel"][:], agg_fm[:, sl2], start=True, stop=False
                    )
                    nc.tensor.matmul(
                        ps[:], w_t[layer, "root"][:], h_fm[:, sl2], start=False, stop=True
                    )
                    nc.scalar.activation(
                        h_next[:, sl2],
                        ps[:],
                        relu if layer < 3 else ident,
                        bias=b_t[layer][:],
                    )

                # share / pool
                h_nm = statep.tile(
                    [128, w_star, F], BF16, tag=f"hnm{layer % 2}", name=f"hnm_{layer}"
                )
                nc.sync.dma_start_transpose(h_nm[:], h_next[:])
                if layer < 3:
                    ag_in = dramp.tile(
                        [128, ls], BF16, name=f"agin_{layer}", tag=f"agin{layer}"
                    )
                    hf = dramp.tile(
                        [rows, F],
                        BF16,
                        name=f"hf_{layer}",
                        tag=f"hf{layer}",
                        addr_space="Shared",
                    )
                    nc.sync.dma_start(ag_in[:], h_nm[:])
                    nc.gpsimd.collective_compute(
                        "AllGather",
                        mybir.AluOpType.bypass,
                        replica_groups=[list(range(N_CORES))],
                        ins=[ag_in[:]],
                        outs=[hf[:]],
                    )
                    gather_src = hf
                    h_fm = h_next
                else:
                    # pooling: pooledT[f, g] += h_nm[:, c, :].T @ B
                    ps_pool = psp.tile([128, ng], F32, tag="pspool")
                    for c in range(w_star):
                        nc.tensor.matmul(
                            ps_pool[:],
                            h_nm[:, c, :],
                            bone_t[:, c * ng : (c + 1) * ng],
                            start=(c == 0),
                            stop=(c == w_star - 1),
                        )
                    pooledT = statep.tile([128, ng], F32, tag="pooledT")
                    nc.scalar.activation(pooledT[:], ps_pool[:], copy_f)
                    ps_head = psp.tile([N_CLASSES, ng], F32, tag="pshead")
                    nc.tensor.matmul(ps_head[:], wlin_t[:], pooledT[:])
                    out_sb = statep.tile([N_CLASSES, ng], F32, tag="outsb")
                    nc.vector.tensor_copy(out_sb[:], ps_head[:])
                    nc.sync.dma_start(out_d[:], out_sb[:])

    nc.compile()
    return nc


def postprocess(results, batch, b_lin, n_graphs):
    """results: list of per-core dicts with 'out_partial' [10, ng]."""
    total = np.zeros_like(np.asarray(results[0]["out_partial"], np.float32))
    for r in results:
        total += np.asarray(r["out_partial"], np.float32)
    cnt = np.bincount(np.asarray(batch, np.int64), minlength=n_graphs).astype(
        np.float32
    )
    cnt = np.maximum(cnt, 1.0)
    logits = total[:, :n_graphs].T / cnt[:, None] + np.asarray(b_lin, np.float32)[None, :]
    return logits.astype(np.float32)


# ----------------------------------------------------------------------------
# harness entry point
# ----------------------------------------------------------------------------
from concourse.bass_utils import run_bass_kernel_spmd

_CACHE = {}


def kernel(x, edge_index, batch,
           W1_rel, b1_rel, W1_root,
           W2_rel, b2_rel, W2_root,
           W3_rel, b3_rel, W3_root,
           W_lin, b_lin):
    params = dict(W1_rel=W1_rel, b1_rel=b1_rel, W1_root=W1_root,
                  W2_rel=W2_rel, b2_rel=b2_rel, W2_root=W2_root,
                  W3_rel=W3_rel, b3_rel=b3_rel, W3_root=W3_root,
                  W_lin=W_lin, b_lin=b_lin)
    n_nodes = int(np.asarray(x).shape[0])
    n_graphs = 64
    meta, in_maps = preprocess(x, edge_index, batch, params, n_nodes, n_graphs)
    key = (meta["w_star"], meta["ls"], meta["rows"])
    if key not in _CACHE:
        _CACHE[key] = build_nc(meta)
    nc = _CACHE[key]
    res = run_bass_kernel_spmd(nc, in_maps, core_ids=list(range(N_CORES)))
    return postprocess(res.results, batch, b_lin, n_graphs)



# revision 3
# speedup vs baseline: 2.2275x; 2.2275x over previous
"""GraphConv GNN kernel for trn2: host preprocessing + bass program builder.

Sharding: nodes (and incident edges, by dst) across 8 cores. Weights
replicated. Three structural optimizations over the dma_gather-everywhere
baseline (which was GpSimd-bound on gather descriptor generation):

- Layer 1: the gather of x[src] is a host-side permutation of an input
  tensor, so it is pre-gathered on the host into a sequential stream and
  DMA'd in chunk order (no dma_gather, no Q7 work).
- Layer 2: real dma_gather from the AllGather'd h1 (unavoidable: h1 is
  computed on device and relu is nonlinear).
- Layer 3 is linear and feeds only mean-pooling, so pooling commutes with
  it: pool(agg3)[g] = sum_u C[u,g] h2[u] with C[u,g] = #edges from node u
  into graph g (host-built count matrix), and pool(h2) uses the batch
  one-hot. Both are small dense matmuls over local node chunks; partial
  sums are combined on the host. This removes the layer-3 gather, its
  one-hot streams, and the second AllGather entirely.
"""

import sys

sys.path.insert(0, "/opt/trn_rl_repo")

import numpy as np
import ml_dtypes

import concourse.bass as bass
import concourse.bacc as bacc
import concourse.tile as tile
import concourse.mybir as mybir
from concourse import library_config

BF16 = mybir.dt.bfloat16
F32 = mybir.dt.float32
I16 = mybir.dt.int16

N_CORES = 8
F = 128
N_CLASSES = 10

# per-window structure: K_LO lo-chunks + K_HI hi-chunks of 128 edges each
K_LO = 6
K_HI = 6
EDGES_PER_HALF = K_LO * 128  # 768
CHUNKS_PER_WIN = K_LO + K_HI
CPO = 32  # gather chunks per dma_gather op (4096 idxs)


def _wrap_idx(idx_flat):
    """idx i -> partition i%16, col i//16; replicated across the 8 Q7 core
    stripes (16 partitions each)."""
    n = idx_flat.shape[0]
    return np.ascontiguousarray(
        np.tile(idx_flat.reshape(n // 16, 16).T.astype(np.int16), (8, 1))
    )


def preprocess(x, edge_index, batch, params, n_nodes, n_graphs):
    """Build per-core inputs + meta for the SPMD program."""
    assert n_nodes % N_CORES == 0
    npc = n_nodes // N_CORES
    src = np.asarray(edge_index[0], np.int64)
    dst = np.asarray(edge_index[1], np.int64)
    batch = np.asarray(batch, np.int64)
    x = np.asarray(x, np.float32)

    half_node = (N_CORES // 2) * npc  # src < half_node -> "lo"

    # sort edges by dst once
    order = np.argsort(dst, kind="stable")
    src_s, dst_s = src[order], dst[order]

    # per-core edge ranges
    core_edge_start = np.searchsorted(dst_s, np.arange(0, n_nodes + 1, npc))

    # --- pass 1: greedy windows per core -> W_k, slots ---
    core_windows = []  # per core: list of (dst_start, dst_end) local
    for k in range(N_CORES):
        e0, e1 = core_edge_start[k], core_edge_start[k + 1]
        dl = dst_s[e0:e1] - k * npc
        sl_lo = src_s[e0:e1] < half_node
        deg_lo = np.bincount(dl[sl_lo], minlength=npc)
        deg_hi = np.bincount(dl[~sl_lo], minlength=npc)
        wins = []
        d = 0
        while d < npc:
            start = d
            lo = hi = 0
            while (
                d < npc
                and d - start < 128
                and lo + deg_lo[d] <= EDGES_PER_HALF
                and hi + deg_hi[d] <= EDGES_PER_HALF
            ):
                lo += deg_lo[d]
                hi += deg_hi[d]
                d += 1
            assert d > start, "single dst exceeds per-window edge budget"
            wins.append((start, d))
        core_windows.append(wins)

    w_star = max(len(w) for w in core_windows)
    w_star = (w_star + 3) // 4 * 4  # LOCAL_SLOTS multiple of 512
    ls = w_star * 128  # LOCAL_SLOTS
    rows = N_CORES * ls
    half_rows = rows // 2
    assert half_rows <= 32768, f"half_rows={half_rows} exceeds int16 idx range"

    # --- slots for every node ---
    slot = np.full(n_nodes, -1, np.int64)
    for k in range(N_CORES):
        for w, (a, b) in enumerate(core_windows[k]):
            d_loc = np.arange(a, b)
            slot[k * npc + d_loc] = w * 128 + (d_loc - a)
    assert (slot >= 0).all()
    owner = np.arange(n_nodes) // npc
    # node-major row index (transpose convention: fm pos s -> (p=s%128, c=s//128),
    # partition-major DRAM -> row = p*W* + c)
    row_of = owner * ls + (slot % 128) * w_star + slot // 128

    x_bf = x.astype(ml_dtypes.bfloat16)

    # --- per-core streams ---
    per_core = []
    for k in range(N_CORES):
        e0, e1 = core_edge_start[k], core_edge_start[k + 1]
        dl = dst_s[e0:e1] - k * npc
        sv = src_s[e0:e1]
        is_lo = sv < half_node
        idx_lo = np.zeros((w_star, EDGES_PER_HALF), np.int64)
        ids_lo = np.full((w_star, EDGES_PER_HALF), -1.0, np.float32)
        src_lo = np.full((w_star, EDGES_PER_HALF), -1, np.int64)
        idx_hi = np.zeros_like(idx_lo)
        ids_hi = np.full_like(ids_lo, -1.0)
        src_hi = np.full_like(src_lo, -1)
        # edges are dst-sorted; window edge groups are contiguous
        wbounds = np.searchsorted(
            dl, [a for a, _ in core_windows[k]] + [npc]
        )
        for w, (a, b) in enumerate(core_windows[k]):
            lo_m = is_lo[wbounds[w] : wbounds[w + 1]]
            e_dst = dl[wbounds[w] : wbounds[w + 1]]
            e_src = sv[wbounds[w] : wbounds[w + 1]]
            for half, m in ((0, lo_m), (1, ~lo_m)):
                r = row_of[e_src[m]] - (0 if half == 0 else half_rows)
                cnt = r.shape[0]
                assert cnt <= EDGES_PER_HALF
                tgt_idx = idx_lo if half == 0 else idx_hi
                tgt_ids = ids_lo if half == 0 else ids_hi
                tgt_src = src_lo if half == 0 else src_hi
                tgt_idx[w, :cnt] = r
                tgt_ids[w, :cnt] = (e_dst[m] - a).astype(np.float32)
                tgt_src[w, :cnt] = e_src[m]

        def _onehot(ids_arr):
            nch = ids_arr.size // 128
            ids_r = ids_arr.reshape(nch, 128)
            oh = (ids_r[:, :, None] == np.arange(128, dtype=np.float32)[None, None, :])
            return np.ascontiguousarray(
                oh.transpose(1, 0, 2).reshape(128, nch * 128).astype(ml_dtypes.bfloat16))

        def _pregather(src_arr):
            # slot j (chunk c=j//128, p=j%128) -> x[src]; layout [128, nch*F]
            flat = src_arr.reshape(-1)
            g = np.zeros((flat.shape[0], F), ml_dtypes.bfloat16)
            v = flat >= 0
            g[v] = x_bf[flat[v]]
            nch = flat.shape[0] // 128
            # [nch, 128, F] -> [128, nch, F]
            return np.ascontiguousarray(
                g.reshape(nch, 128, F).transpose(1, 0, 2).reshape(128, nch * F))

        per_core.append(
            dict(
                idx_lo=_wrap_idx(idx_lo.reshape(-1)),
                idx_hi=_wrap_idx(idx_hi.reshape(-1)),
                s_lo=_onehot(ids_lo.reshape(-1)),
                s_hi=_onehot(ids_hi.reshape(-1)),
                xg_lo=_pregather(src_lo),
                xg_hi=_pregather(src_hi),
            )
        )

    # --- per-(src node, graph) edge-count matrix for the pooled layer-3 ---
    gmax = 64
    c_full = np.zeros((n_nodes, gmax), np.float32)
    np.add.at(c_full, (src, batch[dst]), 1.0)

    def _node_major_64(vals_per_node, k):
        """vals [npc, 64] for core k's local nodes -> [128, w_star*64] in
        node-major chunk layout (row p, block c) = node at slot c*128+p."""
        out = np.zeros((ls, gmax), np.float32)
        g = np.arange(k * npc, (k + 1) * npc)
        out[slot[g]] = vals_per_node
        out = out.reshape(w_star, 128, gmax).transpose(1, 0, 2)
        return np.ascontiguousarray(
            out.reshape(128, w_star * gmax).astype(ml_dtypes.bfloat16))

    in_maps = []
    for k in range(N_CORES):
        g = np.arange(k * npc, (k + 1) * npc)
        x_fm = np.zeros((F, ls), ml_dtypes.bfloat16)
        x_fm[:, slot[g]] = x_bf[g].T
        b_vals = np.zeros((npc, gmax), np.float32)
        b_vals[np.arange(npc), batch[g]] = 1.0
        m = dict(
            x_fm=x_fm,
            b_onehot=_node_major_64(b_vals, k),
            c_onehot=_node_major_64(c_full[g], k),
            idx_lo=per_core[k]["idx_lo"],
            idx_hi=per_core[k]["idx_hi"],
            s_lo=per_core[k]["s_lo"],
            s_hi=per_core[k]["s_hi"],
            xg_lo=per_core[k]["xg_lo"],
            xg_hi=per_core[k]["xg_hi"],
            w1relT=np.ascontiguousarray(params["W1_rel"].T.astype(ml_dtypes.bfloat16)),
            w1rootT=np.ascontiguousarray(
                params["W1_root"].T.astype(ml_dtypes.bfloat16)
            ),
            w2relT=np.ascontiguousarray(params["W2_rel"].T.astype(ml_dtypes.bfloat16)),
            w2rootT=np.ascontiguousarray(
                params["W2_root"].T.astype(ml_dtypes.bfloat16)
            ),
            w3relT=np.ascontiguousarray(params["W3_rel"].T.astype(np.float32)),
            w3rootT=np.ascontiguousarray(
                params["W3_root"].T.astype(np.float32)
            ),
            b1=np.ascontiguousarray(params["b1_rel"].astype(np.float32).reshape(F, 1)),
            b2=np.ascontiguousarray(params["b2_rel"].astype(np.float32).reshape(F, 1)),
            wlinT=np.ascontiguousarray(params["W_lin"].T.astype(np.float32)),
        )
        in_maps.append(m)

    meta = dict(w_star=w_star, ls=ls, rows=rows, half_rows=half_rows, n_graphs=n_graphs)
    return meta, in_maps


def build_nc(meta, n_graphs_pad=64):
    w_star = meta["w_star"]
    ls = meta["ls"]
    rows = meta["rows"]
    half_rows = meta["half_rows"]
    sl_len = w_star * EDGES_PER_HALF  # idxs per stream
    n_chunks = sl_len // 128
    dw = ls // 512  # dense windows
    ng = n_graphs_pad

    nc = bacc.Bacc(
        "TRN2", target_bir_lowering=False, debug=False, num_devices=N_CORES
    )

    # --- I/O ---
    x_fm_d = nc.dram_tensor("x_fm", [F, ls], BF16, kind="ExternalInput")
    bone_d = nc.dram_tensor("b_onehot", [128, w_star * 64], BF16, kind="ExternalInput")
    cone_d = nc.dram_tensor("c_onehot", [128, w_star * 64], BF16, kind="ExternalInput")
    idx_d = {
        "lo": nc.dram_tensor("idx_lo", [128, sl_len // 16], I16, kind="ExternalInput"),
        "hi": nc.dram_tensor("idx_hi", [128, sl_len // 16], I16, kind="ExternalInput"),
    }
    s_d = {
        "lo": nc.dram_tensor("s_lo", [128, n_chunks * 128], BF16, kind="ExternalInput"),
        "hi": nc.dram_tensor("s_hi", [128, n_chunks * 128], BF16, kind="ExternalInput"),
    }
    xg_d = {
        "lo": nc.dram_tensor("xg_lo", [128, n_chunks * F], BF16, kind="ExternalInput"),
        "hi": nc.dram_tensor("xg_hi", [128, n_chunks * F], BF16, kind="ExternalInput"),
    }
    w_d = {}
    for l in (1, 2):
        for p in ("rel", "root"):
            w_d[l, p] = nc.dram_tensor(f"w{l}{p}T", [F, F], BF16, kind="ExternalInput")
    w3_d = {
        "rel": nc.dram_tensor("w3relT", [F, F], F32, kind="ExternalInput"),
        "root": nc.dram_tensor("w3rootT", [F, F], F32, kind="ExternalInput"),
    }
    b_d = {l: nc.dram_tensor(f"b{l}", [F, 1], F32, kind="ExternalInput") for l in (1, 2)}
    wlin_d = nc.dram_tensor("wlinT", [F, N_CLASSES], F32, kind="ExternalInput")
    out_d = nc.dram_tensor("out_partial", [N_CLASSES, ng], F32, kind="ExternalOutput")

    relu = mybir.ActivationFunctionType.Relu
    copy_f = mybir.ActivationFunctionType.Copy

    n_ops = (n_chunks + CPO - 1) // CPO

    with tile.TileContext(nc) as tc:
        with (
            tc.tile_pool(name="const", bufs=1) as constp,
            tc.tile_pool(name="state", bufs=1) as statep,
            tc.tile_pool(name="gpool", bufs=2) as gpool,
            tc.tile_pool(name="spool", bufs=8) as spool,
            tc.tile_pool(name="psa", bufs=2, space="PSUM") as psa,
            tc.tile_pool(name="psd", bufs=2, space="PSUM") as psd,
            tc.tile_pool(name="psp", bufs=1, space="PSUM") as psp,
            tc.tile_pool(name="dram", bufs=1, space="DRAM") as dramp,
        ):
            nc.gpsimd.load_library(library_config.mlp)

            # ---- load constants ----
            bone_t = constp.tile([128, w_star * 64], BF16)
            nc.sync.dma_start(bone_t[:], bone_d[:])
            cone_t = constp.tile([128, w_star * 64], BF16)
            nc.sync.dma_start(cone_t[:], cone_d[:])
            idx_t = {}
            for h in ("lo", "hi"):
                it = constp.tile([128, sl_len // 16], I16, name=f"idx_{h}")
                nc.sync.dma_start(it[:], idx_d[h][:])
                idx_t[h] = it
            w_t = {}
            for key, d in w_d.items():
                wt = constp.tile([F, F], BF16, name=f"w_{key[0]}_{key[1]}")
                nc.sync.dma_start(wt[:], d[:])
                w_t[key] = wt
            w3_t = {}
            for p, d in w3_d.items():
                wt = constp.tile([F, F], F32, name=f"w3_{p}")
                nc.sync.dma_start(wt[:], d[:])
                w3_t[p] = wt
            b_t = {}
            for l, d in b_d.items():
                bt = constp.tile([F, 1], F32, name=f"b_{l}")
                nc.sync.dma_start(bt[:], d[:])
                b_t[l] = bt
            wlin_t = constp.tile([F, N_CLASSES], F32)
            nc.sync.dma_start(wlin_t[:], wlin_d[:])

            x_fm_t = statep.tile([F, ls], BF16, tag="h0")
            nc.sync.dma_start(x_fm_t[:], x_fm_d[:])

            # ---- layers 1 (pre-gathered stream) and 2 (dma_gather) ----
            h_fm = x_fm_t
            gather_src = None  # layer-2 source (set after AllGather)
            for layer in (1, 2):
                g_tiles = {"lo": [], "hi": []}
                s_tiles = {"lo": [], "hi": []}
                for o in range(n_ops):
                    c0 = o * CPO
                    c1 = min(n_chunks, c0 + CPO)
                    nch = c1 - c0
                    for h in ("lo", "hi"):
                        st_ = spool.tile(
                            [128, nch * 128],
                            BF16,
                            name=f"sg_{layer}_{h}_{o}",
                            tag=f"sg_{h}",
                            bufs=2,
                        )
                        nc.sync.dma_start(
                            st_[:], s_d[h][:, c0 * 128 : c1 * 128]
                        )
                        s_tiles[h].append(st_)
                        gt = gpool.tile(
                            [128, nch, F],
                            BF16,
                            name=f"g_{layer}_{h}_{o}",
                            tag=f"g_{h}",
                            padded_shape=[128, CPO, F],
                        )
                        if layer == 1:
                            nc.sync.dma_start(
                                gt[:], xg_d[h][:, c0 * F : c1 * F]
                            )
                        else:
                            src_ap = (
                                gather_src[0:half_rows, :]
                                if h == "lo"
                                else gather_src[half_rows:rows, :]
                            )
                            nidx = nch * 128
                            nc.gpsimd.dma_gather(
                                gt[:],
                                src_ap,
                                idx_t[h][:, c0 * 8 : c1 * 8],
                                nidx,
                                nidx,
                                F,
                                single_packet=False,
                            )
                        g_tiles[h].append(gt)

                # aggregation windows
                agg_fm = statep.tile([F, ls], BF16, tag=f"agg{layer}", name=f"agg_{layer}")
                for w in range(w_star):
                    ps = psa.tile([128, 128], F32, name=f"psagg_{layer}_{w}", tag="psagg")
                    for j in range(CHUNKS_PER_WIN):
                        h = "lo" if j < K_LO else "hi"
                        cc = w * K_LO + (j % K_LO)
                        o, sl_ = cc // CPO, cc % CPO
                        nc.tensor.matmul(
                            ps[:],
                            g_tiles[h][o][:, sl_, :],
                            s_tiles[h][o][:, sl_ * 128 : (sl_ + 1) * 128],
                            start=(j == 0),
                            stop=(j == CHUNKS_PER_WIN - 1),
                        )
                    nc.scalar.activation(
                        agg_fm[:, w * 128 : (w + 1) * 128], ps[:], copy_f
                    )

                # dense
                h_next = statep.tile([F, ls], BF16, tag=f"h{layer}", name=f"h_{layer}")
                for d in range(dw):
                    ps = psd.tile([128, 512], F32, name=f"psd_{layer}_{d}", tag="psd")
                    sl2 = slice(d * 512, (d + 1) * 512)
                    nc.tensor.matmul(
                        ps[:], w_t[layer, "rel"][:], agg_fm[:, sl2], start=True, stop=False
                    )
                    nc.tensor.matmul(
                        ps[:], w_t[layer, "root"][:], h_fm[:, sl2], start=False, stop=True
                    )
                    nc.scalar.activation(
                        h_next[:, sl2],
                        ps[:],
                        relu,
                        bias=b_t[layer][:],
                    )

                if layer == 1:
                    # share h1 for the layer-2 gather
                    h_nm = statep.tile(
                        [128, w_star, F], BF16, tag="h1nm", name="hnm_1"
                    )
                    nc.sync.dma_start_transpose(h_nm[:], h_next[:])
                    ag_in = dramp.tile([128, ls], BF16, name="agin_1", tag="agin1")
                    hf = dramp.tile(
                        [rows, F],
                        BF16,
                        name="hf_1",
                        tag="hf1",
                        addr_space="Shared",
                    )
                    nc.sync.dma_start(ag_in[:], h_nm[:])
                    nc.gpsimd.collective_compute(
                        "AllGather",
                        mybir.AluOpType.bypass,
                        replica_groups=[list(range(N_CORES))],
                        ins=[ag_in[:]],
                        outs=[hf[:]],
                    )
                    gather_src = hf
                    h_fm = h_next
                else:
                    h2_fm = h_next

            # ---- pooled layer 3: pool(agg3) = C^T h2, pool(h2) = B^T h2 ----
            h2_nm = statep.tile([128, w_star, F], BF16, tag="h2nm", name="hnm_2")
            nc.sync.dma_start_transpose(h2_nm[:], h2_fm[:])

            ps_pa = psp.tile([128, ng], F32, tag="pspa")
            ps_ph = psp.tile([128, ng], F32, tag="psph")
            for c in range(w_star):
                nc.tensor.matmul(
                    ps_pa[:],
                    h2_nm[:, c, :],
                    cone_t[:, c * ng : (c + 1) * ng],
                    start=(c == 0),
                    stop=(c == w_star - 1),
                )
            for c in range(w_star):
                nc.tensor.matmul(
                    ps_ph[:],
                    h2_nm[:, c, :],
                    bone_t[:, c * ng : (c + 1) * ng],
                    start=(c == 0),
                    stop=(c == w_star - 1),
                )
            pa_sb = statep.tile([128, ng], F32, tag="pasb")
            ph_sb = statep.tile([128, ng], F32, tag="phsb")
            nc.scalar.activation(pa_sb[:], ps_pa[:], copy_f)
            nc.scalar.activation(ph_sb[:], ps_ph[:], copy_f)

            # z3p = W3_rel @ pool(agg3) + W3_root @ pool(h2)   [128, ng] f32
            ps_z = psp.tile([128, ng], F32, tag="psz")
            nc.tensor.matmul(ps_z[:], w3_t["rel"][:], pa_sb[:], start=True, stop=False)
            nc.tensor.matmul(ps_z[:], w3_t["root"][:], ph_sb[:], start=False, stop=True)
            z_sb = statep.tile([128, ng], F32, tag="zsb")
            nc.vector.tensor_copy(z_sb[:], ps_z[:])

            ps_head = psp.tile([N_CLASSES, ng], F32, tag="pshead")
            nc.tensor.matmul(ps_head[:], wlin_t[:], z_sb[:])
            out_sb = statep.tile([N_CLASSES, ng], F32, tag="outsb")
            nc.vector.tensor_copy(out_sb[:], ps_head[:])
            nc.sync.dma_start(out_d[:], out_sb[:])

    nc.compile()
    return nc


def postprocess(results, batch, b3_rel, W_lin, b_lin, n_graphs):
    """results: list of per-core dicts with 'out_partial' [10, ng]."""
    total = np.zeros_like(np.asarray(results[0]["out_partial"], np.float32))
    for r in results:
        total += np.asarray(r["out_partial"], np.float32)
    cnt = np.bincount(np.asarray(batch, np.int64), minlength=n_graphs).astype(
        np.float32
    )
    cnt = np.maximum(cnt, 1.0)
    W_lin = np.asarray(W_lin, np.float32)
    # per-node bias b3 pools to cnt[g]*b3; after the mean it is just b3.
    logits = (
        total[:, :n_graphs].T / cnt[:, None]
        + np.asarray(b3_rel, np.float32)[None, :] @ W_lin.T
        + np.asarray(b_lin, np.float32)[None, :]
    )
    return logits.astype(np.float32)


# ----------------------------------------------------------------------------
# harness entry point
# ----------------------------------------------------------------------------
from concourse.bass_utils import run_bass_kernel_spmd

_CACHE = {}


def kernel(x, edge_index, batch,
           W1_rel, b1_rel, W1_root,
           W2_rel, b2_rel, W2_root,
           W3_rel, b3_rel, W3_root,
           W_lin, b_lin):
    params = dict(W1_rel=W1_rel, b1_rel=b1_rel, W1_root=W1_root,
                  W2_rel=W2_rel, b2_rel=b2_rel, W2_root=W2_root,
                  W3_rel=W3_rel, b3_rel=b3_rel, W3_root=W3_root,
                  W_lin=W_lin, b_lin=b_lin)
    n_nodes = int(np.asarray(x).shape[0])
    n_graphs = 64
    meta, in_maps = preprocess(x, edge_index, batch, params, n_nodes, n_graphs)
    key = (meta["w_star"], meta["ls"], meta["rows"])
    if key not in _CACHE:
        _CACHE[key] = build_nc(meta)
    nc = _CACHE[key]
    res = run_bass_kernel_spmd(nc, in_maps, core_ids=list(range(N_CORES)))
    return postprocess(res.results, batch, b3_rel, W_lin, b_lin, n_graphs)
